# revision 1
# baseline (speedup 1.0000x reference)
"""Trainium2 Bass kernel for nn_DegreePrediction (RBC via batched Perron vectors).

Math: M[s,t] = weights_r*r_zeros + r_const is positive column-stochastic
(columns sum to 1), so its eigenvalue-1 right eigenvector is the Perron
vector, and the reference formula  rbc[n] = sum_{s,t} T[s,t]/v[s,t,s] * v[s,t,n]
is invariant to the scale of v.  Columns of M^4 converge to v at rate
lambda2^4 (lambda2 <= ~0.09 here), so two batched 64x64 matmul squarings
per (s,t) pair give v to fp32 accuracy.

Sharding: the 4096 (s,t) pairs are split by s across 8 cores (512 pairs
each).  Each core computes a partial 64-vector; the host sums the 8 partials.

Device pipeline per chunk of 8 matrices: all PE transposes are "double
transposes" ([64,128] input -> [128,64] output) at tile_position (0,0) —
transpose-mode matmuls crash the PE when consecutive ops switch row
groups, and the Tile scheduler is free to interleave transposes from
different chunks, so every transpose must use the same quadrant.  The
double transpose also lands the pair 2-stacked across partition halves,
which is exactly the layout the (quadrant-alternating, crash-safe)
squaring matmuls need.  DVE/ACT split the PSUM->SBUF copies (partition-
shifted where the half structure must be rebuilt); the per-pair
denominators v[s] are gathered on DVE with a mask in the transposed-V
layout because PE ones-matmuls with a 128-wide lhsT lose ~11 bits.
"""

import numpy as np

_N = 64
_NCORES = 8
_NP = 512          # pairs per core
_NCHUNK = 64       # chunks of 8 pairs

_cached = {}


def _build_program():
    import concourse.tile as tile
    from concourse import bacc, mybir
    from contextlib import ExitStack

    f32 = mybir.dt.float32
    nc = bacc.Bacc("TRN2", target_bir_lowering=False, debug=False)
    m_in = nc.dram_tensor("m", [_NP, _N, _N], f32, kind="ExternalInput").ap()
    mask_in = nc.dram_tensor("mask", [128, 4, _N], f32, kind="ExternalInput").ap()
    xpp_in = nc.dram_tensor("xpp", [128, 4], f32, kind="ExternalInput").ap()
    wpp_in = nc.dram_tensor("wpp", [128, 4], f32, kind="ExternalInput").ap()
    rpp_in = nc.dram_tensor("rpp", [128, 4], f32, kind="ExternalInput").ap()
    ident_in = nc.dram_tensor("ident", [_N, _N], f32, kind="ExternalInput").ap()
    out_dram = nc.dram_tensor("out", [_N, 1], f32, kind="ExternalOutput").ap()

    with tile.TileContext(nc) as tc:
        with ExitStack() as ctx:
            consts = ctx.enter_context(tc.tile_pool(name="consts", bufs=1))
            work = ctx.enter_context(tc.tile_pool(name="work", bufs=4))
            psum = ctx.enter_context(tc.tile_pool(name="psum", bufs=2, space="PSUM"))

            ident = consts.tile([_N, _N], f32)
            nc.sync.dma_start(out=ident[:, :], in_=ident_in[:, :])
            mask_sb = consts.tile([128, 4, _N], f32)
            nc.sync.dma_start(out=mask_sb[:, :, :], in_=mask_in[:, :, :])
            xpp_sb = consts.tile([128, 4], f32)
            nc.sync.dma_start(out=xpp_sb[:, :], in_=xpp_in[:, :])
            wpp_sb = consts.tile([128, 4], f32)
            nc.sync.dma_start(out=wpp_sb[:, :], in_=wpp_in[:, :])
            rpp_sb = consts.tile([128, 4], f32)
            nc.sync.dma_start(out=rpp_sb[:, :], in_=rpp_in[:, :])
            ones = consts.tile([128, 1], f32)
            nc.vector.memset(ones[:, :], 1.0)
            v_sb = consts.tile([128, _NCHUNK, 4], f32)

            def split2(t):
                """[64|128, 8, 64] tile -> (even-slot view, odd-slot view)."""
                r = t[:, :, :].rearrange("p (c two) j -> p c two j", two=2)
                return r[:, :, 0, :], r[:, :, 1, :]

            for k in range(_NCHUNK):
                mc64 = work.tile([_N, 8, _N], f32, tag="mc64")
                nc.sync.dma_start(
                    out=mc64[:, :, :],
                    in_=m_in[8 * k: 8 * k + 8, :, :].rearrange("p i j -> i p j"))
                # 2-stacked copy for matmul rhs: mcS[64h+i, d] = M_{8k+2d+h}
                mcS = work.tile([128, 4, _N], f32, tag="mcS")
                mc_ev, mc_od = split2(mc64)
                nc.scalar.copy(out=mcS[0:64, :, :], in_=mc_ev)
                nc.vector.tensor_copy(out=mcS[64:128, :, :], in_=mc_od)
                # double transposes: pt[:, d] = [M_{2d}^T ; M_{2d+1}^T] stacked
                pt = psum.tile([128, 4, _N], f32, tag="pt")
                for d in range(4):
                    nc.tensor.transpose(
                        out=pt[:, d, :],
                        in_=mc64[:, 2 * d:2 * d + 2, :].rearrange("p a j -> p (a j)"),
                        identity=ident[:, :])
                mtS = work.tile([128, 4, _N], f32, tag="mtS")
                nc.vector.tensor_copy(out=mtS[:, :, :], in_=pt[:, :, :])
                # M^2, quadrant-alternating (safe for regular matmuls)
                pp = psum.tile([128, 4, _N], f32, tag="pp")
                for d in range(4):
                    for h in (0, 1):
                        b = 64 * h
                        nc.tensor.matmul(
                            out=pp[b:b + 64, d, :],
                            lhsT=mtS[b:b + 64, d, :],
                            rhs=mcS[b:b + 64, d, :],
                            start=True, stop=True)
                p1S = work.tile([128, 4, _N], f32, tag="p1S")
                nc.scalar.copy(out=p1S[:, :, :], in_=pp[:, :, :])
                p1_64 = work.tile([_N, 8, _N], f32, tag="p1_64")
                p1_ev, p1_od = split2(p1_64)
                nc.scalar.copy(out=p1_ev, in_=pp[0:64, :, :])
                nc.vector.tensor_copy(out=p1_od, in_=pp[64:128, :, :])
                pt2 = psum.tile([128, 4, _N], f32, tag="pt2")
                for d in range(4):
                    nc.tensor.transpose(
                        out=pt2[:, d, :],
                        in_=p1_64[:, 2 * d:2 * d + 2, :].rearrange("p a j -> p (a j)"),
                        identity=ident[:, :])
                q1S = work.tile([128, 4, _N], f32, tag="q1S")
                nc.vector.tensor_copy(out=q1S[:, :, :], in_=pt2[:, :, :])
                pp2 = psum.tile([128, 4, _N], f32, tag="pp2")
                for d in range(4):
                    for h in (0, 1):
                        b = 64 * h
                        nc.tensor.matmul(
                            out=pp2[b:b + 64, d, :],
                            lhsT=q1S[b:b + 64, d, :],
                            rhs=p1S[b:b + 64, d, :],
                            start=True, stop=True)
                nc.vector.tensor_reduce(
                    out=v_sb[:, k, :], in_=pp2[:, :, :],
                    axis=mybir.AxisListType.X, op=mybir.AluOpType.add)

            # ---- tail ----
            v_flat = v_sb[:, :, :].rearrange("p a b -> p (a b)")  # [128, 256]
            # V rows to partitions 0-63, then 4 (0,0) double-transposes
            v64 = consts.tile([_N, 2, 256], f32)
            nc.gpsimd.tensor_copy(out=v64[:, 0, :], in_=v_flat[0:64, :])
            nc.gpsimd.tensor_copy(out=v64[:, 1, :], in_=v_flat[64:128, :])
            pvt = psum.tile([128, 4, _N], f32, tag="pp")
            for h in (0, 1):
                for g in (0, 1):
                    j = 2 * h + g
                    nc.tensor.transpose(
                        out=pvt[:, j, :],
                        in_=v64[:, h, 128 * g:128 * g + 128],
                        identity=ident[:, :])
            vt = consts.tile([128, 4, _N], f32)
            nc.vector.tensor_copy(out=vt[:, :, :], in_=pvt[:, :, :])
            # denominators v[s] via VT-layout mask gather on DVE (exact fp32;
            # a PE ones-matmul with 128-wide lhsT loses ~11 bits)
            maskv = consts.tile([128, 4, _N], f32)
            nc.vector.tensor_mul(out=maskv[:, :, :], in0=vt[:, :, :],
                                 in1=mask_sb[:, :, :])
            d_sb = consts.tile([128, 4], f32)
            nc.vector.tensor_reduce(
                out=d_sb[:, :], in_=maskv[:, :, :],
                axis=mybir.AxisListType.X, op=mybir.AluOpType.add)
            dinv = consts.tile([128, 4], f32)
            nc.vector.reciprocal(out=dinv[:, :], in_=d_sb[:, :])
            tpp = consts.tile([128, 4], f32)
            nc.vector.tensor_mul(out=tpp[:, :], in0=xpp_sb[:, :], in1=wpp_sb[:, :])
            nc.vector.tensor_mul(out=tpp[:, :], in0=tpp[:, :], in1=rpp_sb[:, :])
            u = consts.tile([128, 4], f32)
            nc.vector.tensor_mul(out=u[:, :], in0=tpp[:, :], in1=dinv[:, :])
            prbc = psum.tile([_N, 1], f32, tag="pt2")
            for j in range(4):
                nc.tensor.matmul(
                    out=prbc[:, :], lhsT=vt[:, j, :], rhs=u[:, j:j + 1],
                    start=(j == 0), stop=(j == 3))
            out_sb = consts.tile([_N, 1], f32)
            nc.vector.tensor_copy(out=out_sb[:, :], in_=prbc[:, :])
            nc.sync.dma_start(out=out_dram[:, :], in_=out_sb[:, :])
    nc.compile()
    return nc


def _get_program():
    if "nc" not in _cached:
        _cached["nc"] = _build_program()
    return _cached["nc"]


def _pair_of(h, f):
    """Local pair id for half h, V-free-index f (f = 4*chunk + dslot)."""
    return 8 * (f >> 2) + 2 * (f & 3) + h


def _host_layouts(x, weights_t, r_const):
    """Per-core gathers: xpp/wpp/rpp [128,4] pairs-on-partitions, mask [128,256]."""
    Q = np.arange(128)[:, None]
    J = np.arange(4)[None, :]
    h = J >> 1
    g = J & 1
    f = 128 * g + Q
    p = _pair_of(h, f)                      # local pair id [128, 4]
    s_loc = p >> 6
    t = p & 63
    F = np.arange(256)
    outs = []
    for c in range(_NCORES):
        s_glob = 8 * c + s_loc
        xpp = np.ascontiguousarray(x[s_glob, t], np.float32)
        wpp = np.ascontiguousarray(weights_t[s_glob, t], np.float32)
        rpp = np.ascontiguousarray(r_const[s_glob, t, s_glob, s_glob], np.float32)
        # maskT[q, j, i] = 1 iff i == s_glob(pair at VT position (q, j))
        mask = np.zeros((128, 4, _N), np.float32)
        for j in range(4):
            hh = j >> 1
            ff = 128 * (j & 1) + np.arange(128)
            pl = _pair_of(hh, ff)
            sg = 8 * c + (pl >> 6)
            mask[np.arange(128), j, sg] = 1.0
        outs.append((xpp, wpp, rpp, mask))
    return outs


def kernel(x, weights_t, weights_r, r_zeros, r_const):
    from concourse.bass_utils import run_bass_kernel_spmd

    x = np.asarray(x, np.float32)
    weights_t = np.asarray(weights_t, np.float32)
    r_const = np.asarray(r_const, np.float32)
    r_zeros_np = np.asarray(r_zeros)
    if np.any(r_zeros_np):
        M_all = (np.asarray(weights_r, np.float32) * r_zeros_np.astype(np.float32)
                 + r_const).reshape(_N * _N, _N, _N)
    else:
        M_all = r_const.reshape(_N * _N, _N, _N)

    nc = _get_program()
    ident_np = np.eye(_N, dtype=np.float32)
    layouts = _host_layouts(x, weights_t, r_const)
    in_maps = []
    for c in range(_NCORES):
        xpp, wpp, rpp, mask = layouts[c]
        in_maps.append({
            "m": np.ascontiguousarray(M_all[_NP * c:_NP * (c + 1)], np.float32),
            "mask": mask,
            "xpp": xpp,
            "wpp": wpp,
            "rpp": rpp,
            "ident": ident_np,
        })
    res = run_bass_kernel_spmd(nc, in_maps, core_ids=list(range(_NCORES)))
    parts = np.stack([r["out"][:, 0] for r in res.results])  # [8, 64]
    return parts.sum(axis=0, dtype=np.float64).astype(np.float32)



# revision 5
# speedup vs baseline: 3.8187x; 3.8187x over previous
"""Trainium2 Bass kernel for nn_DegreePrediction (RBC via batched Perron vectors).

Math: M[s,t] = weights_r*r_zeros + r_const is positive column-stochastic
(columns sum to 1); its eigenvalue-1 right eigenvector is the Perron
vector and rbc[n] = sum_{s,t} T[s,t]/v[s,t,s] * v[s,t,n] is scale-free in
v.  v ~= M^2 @ ones to ~lambda2^2 ~ 0.4% << the 2e-2 gate, so two batched
mat-vec sweeps suffice (no squarings).

Layout trick: each core's 512 matrices are uploaded TRANSPOSED in bf16,
two per 128-partition stack: MT[j+64h, 64q+i] = M_{2q+h}[i,j].  Then with
lhsT = a [128,128] MT block (stationary, FWL bf16 weight load) both sweeps
keep their results in the PARTITION dim:
  pass A: rhs = ones-blocks [128,2]      -> out[m,n] = rowsums w_p[m]
  pass B: rhs = block-diag w cols [128,4] -> out[m,n] = v_p[m]
so no transposes anywhere.  PE cost is ~LDWEIGHTS-bound (256 loads),
overlapping the 4MB bf16 DMA stream.  Tail: denominators v_p[s_p] via a
mask multiply + ones-matmul row, reciprocal, coefficient broadcast via
K=1 outer-product matmuls, then one weighted free-dim reduce.

Sharding: pairs split by s across 8 cores; host sums the 8 partial
64-vectors.
"""

import numpy as np

_N = 64
_NCORES = 8
_NP = 512          # pairs per core
_NQ = 128          # double-stacks (4 pairs each)
_NCHUNK = 8        # DMA chunks of MT

_cached = {}


def _build_program():
    import concourse.tile as tile
    from concourse import bacc, mybir
    from contextlib import ExitStack

    f32 = mybir.dt.float32
    bf16 = mybir.dt.bfloat16
    nc = bacc.Bacc("TRN2", target_bir_lowering=False, debug=False)
    mt_in = nc.dram_tensor("mt", [128, _NQ * 128], bf16, kind="ExternalInput").ap()
    maskd_in = nc.dram_tensor("maskd", [128, _NP], f32, kind="ExternalInput").ap()
    tmt_in = nc.dram_tensor("tmt", [1, _NP], f32, kind="ExternalInput").ap()
    tmb_in = nc.dram_tensor("tmb", [1, _NP], f32, kind="ExternalInput").ap()
    out_dram = nc.dram_tensor("out", [_N, 1], f32, kind="ExternalOutput").ap()

    CW = _NQ * 128 // _NCHUNK          # MT cols per DMA chunk (2048)
    QPC = _NQ // _NCHUNK               # double-stacks per chunk (16)

    with tile.TileContext(nc) as tc:
        with ExitStack() as ctx:
            consts = ctx.enter_context(tc.tile_pool(name="consts", bufs=1))
            psum = ctx.enter_context(tc.tile_pool(name="psum", bufs=1, space="PSUM"))

            # ---- constants / inputs ----
            maskd = consts.tile([128, _NP], f32)
            nc.sync.dma_start(out=maskd[:, :], in_=maskd_in[:, :])
            tmt = consts.tile([1, _NP], f32)
            nc.sync.dma_start(out=tmt[:, :], in_=tmt_in[:, :])
            tmb = consts.tile([1, _NP], f32)
            nc.sync.dma_start(out=tmb[:, :], in_=tmb_in[:, :])

            ones2 = consts.tile([128, 2], bf16)
            nc.vector.memset(ones2[:, :], 0.0)
            nc.vector.memset(ones2[0:64, 0:1], 1.0)
            nc.vector.memset(ones2[64:128, 1:2], 1.0)
            ones128 = consts.tile([128, 1], f32)
            nc.vector.memset(ones128[:, :], 1.0)
            etop = consts.tile([1, 128], f32)
            nc.vector.memset(etop[:, :], 0.0)
            nc.vector.memset(etop[0:1, 0:64], 1.0)
            ebot = consts.tile([1, 128], f32)
            nc.vector.memset(ebot[:, :], 0.0)
            nc.vector.memset(ebot[0:1, 64:128], 1.0)

            L = consts.tile([128, 4 * _NQ], bf16)
            nc.vector.memset(L[:, :], 0.0)

            # ---- stream MT in chunks ----
            mtc = []
            for d in range(_NCHUNK):
                t = consts.tile([128, CW], bf16, tag=f"mt{d}")
                nc.sync.dma_start(out=t[:, :], in_=mt_in[:, d * CW:(d + 1) * CW])
                mtc.append(t)

            # ---- pass A: rowsums w into partition dim ----
            WW = psum.tile([128, 2 * _NQ], f32, tag="WW")
            for Q in range(_NQ):
                d, r = Q // QPC, Q % QPC
                nc.tensor.matmul(
                    out=WW[:, 2 * Q:2 * Q + 2],
                    lhsT=mtc[d][:, 128 * r:128 * r + 128],
                    rhs=ones2[:, :], start=True, stop=True)

            # ---- build block-diag L from WW (4 strided copies) ----
            WWv = WW[:, :].rearrange("p (q two) -> p q two", two=2)
            Lv = L[:, :].rearrange("p (q four) -> p q four", four=4)
            nc.vector.tensor_copy(out=Lv[0:64, :, 0], in_=WWv[0:64, :, 0])
            nc.vector.tensor_copy(out=Lv[64:128, :, 1], in_=WWv[0:64, :, 1])
            nc.vector.tensor_copy(out=Lv[0:64, :, 2], in_=WWv[64:128, :, 0])
            nc.vector.tensor_copy(out=Lv[64:128, :, 3], in_=WWv[64:128, :, 1])

            # ---- pass B: v_p into partition dim, pair p <-> column p ----
            VV = psum.tile([128, _NP], f32, tag="VV")
            for Q in range(_NQ):
                d, r = Q // QPC, Q % QPC
                nc.tensor.matmul(
                    out=VV[:, 4 * Q:4 * Q + 4],
                    lhsT=mtc[d][:, 128 * r:128 * r + 128],
                    rhs=L[:, 4 * Q:4 * Q + 4], start=True, stop=True)

            # ---- tail ----
            dmm = consts.tile([128, _NP], f32)
            nc.vector.tensor_mul(out=dmm[:, :], in0=VV[:, :], in1=maskd[:, :])
            DPS = psum.tile([1, _NP], f32, tag="DPS")
            nc.tensor.matmul(out=DPS[:, :], lhsT=ones128[:, :], rhs=dmm[:, :],
                             start=True, stop=True)
            dinv = consts.tile([1, _NP], f32)
            nc.vector.reciprocal(out=dinv[:, :], in_=DPS[:, :])
            ct = consts.tile([1, _NP], f32)
            nc.vector.tensor_mul(out=ct[:, :], in0=tmt[:, :], in1=dinv[:, :])
            cb = consts.tile([1, _NP], f32)
            nc.vector.tensor_mul(out=cb[:, :], in0=tmb[:, :], in1=dinv[:, :])
            CB = psum.tile([128, _NP], f32, tag="CB")
            nc.tensor.matmul(out=CB[:, :], lhsT=etop[:, :], rhs=ct[:, :],
                             start=True, stop=False)
            nc.tensor.matmul(out=CB[:, :], lhsT=ebot[:, :], rhs=cb[:, :],
                             start=False, stop=True)
            cbs = consts.tile([128, _NP], f32)
            nc.scalar.copy(out=cbs[:, :], in_=CB[:, :])
            vc = consts.tile([128, _NP], f32)
            nc.vector.tensor_mul(out=vc[:, :], in0=VV[:, :], in1=cbs[:, :])
            r1 = consts.tile([128, 1], f32)
            nc.vector.tensor_reduce(
                out=r1[:, :], in_=vc[:, :],
                axis=mybir.AxisListType.X, op=mybir.AluOpType.add)
            r1lo = consts.tile([_N, 1], f32)
            nc.scalar.copy(out=r1lo[:, :], in_=r1[64:128, :])
            out_sb = consts.tile([_N, 1], f32)
            nc.vector.tensor_add(out=out_sb[:, :], in0=r1[0:64, :],
                                 in1=r1lo[:, :])
            nc.sync.dma_start(out=out_dram[:, :], in_=out_sb[:, :])
    nc.compile()
    return nc


def _get_program():
    if "nc" not in _cached:
        _cached["nc"] = _build_program()
    return _cached["nc"]


def _build_in_maps(x, weights_t, r_const):
    """Host-side layouts for all 8 cores."""
    import ml_dtypes

    M_all = r_const.reshape(_N * _N, _N, _N)
    i = np.arange(_N)
    r_diag = r_const[i[:, None], i[None, :], i[:, None], i[:, None]]
    T_full = (x * weights_t * r_diag).astype(np.float32)      # [64, 64]

    p = np.arange(_NP)
    b = (p >> 1) & 1                                          # stack-half of pair
    s_loc = p >> 6
    t_loc = p & 63

    in_maps = []
    for c in range(_NCORES):
        Mc = np.asarray(M_all[_NP * c:_NP * (c + 1)], np.float32)  # (p,i,j)
        # MT[j+64h, 64(2Q+b)+i] = Mc[4Q+2b+h, i, j]
        mt = (Mc.reshape(_NQ, 2, 2, _N, _N)       # (Q, b, h, i, j)
              .transpose(2, 4, 0, 1, 3)           # (h, j, Q, b, i)
              .reshape(128, _NQ * 128))
        mt = np.ascontiguousarray(mt).astype(ml_dtypes.bfloat16)

        maskd = np.zeros((128, _NP), np.float32)
        maskd[64 * b + 8 * c + s_loc, p] = 1.0   # v's node index is GLOBAL s

        Tp = T_full[8 * c + s_loc, t_loc]                     # [512]
        tmt = np.where(b == 0, Tp, 0.0).astype(np.float32)[None, :]
        tmb = np.where(b == 1, Tp, 0.0).astype(np.float32)[None, :]

        in_maps.append({"mt": mt, "maskd": maskd,
                        "tmt": np.ascontiguousarray(tmt),
                        "tmb": np.ascontiguousarray(tmb)})
    return in_maps


def kernel(x, weights_t, weights_r, r_zeros, r_const):
    from concourse.bass_utils import run_bass_kernel_spmd

    x = np.asarray(x, np.float32)
    weights_t = np.asarray(weights_t, np.float32)
    r_const = np.asarray(r_const, np.float32)
    r_zeros_np = np.asarray(r_zeros)
    if np.any(r_zeros_np):
        r_const = (np.asarray(weights_r, np.float32)
                   * r_zeros_np.astype(np.float32) + r_const)

    nc = _get_program()
    in_maps = _build_in_maps(x, weights_t, r_const)
    res = run_bass_kernel_spmd(nc, in_maps, core_ids=list(range(_NCORES)))
    parts = np.stack([r["out"][:, 0] for r in res.results])  # [8, 64]
    return parts.sum(axis=0, dtype=np.float64).astype(np.float32)


# revision 10
# speedup vs baseline: 3.8419x; 1.0061x over previous
"""Trainium2 Bass kernel for nn_DegreePrediction (RBC via batched Perron vectors).

Math: M[s,t] = weights_r*r_zeros + r_const is positive column-stochastic
(columns sum to 1); its eigenvalue-1 right eigenvector is the Perron
vector and rbc[n] = sum_{s,t} T[s,t]/v[s,t,s] * v[s,t,n] is scale-free in
v.  v ~= M^2 @ ones to ~lambda2^2 ~ 0.4% << the 2e-2 gate, so two batched
mat-vec sweeps suffice (no squarings, no transposes).

Layout trick: each core's 512 matrices are uploaded TRANSPOSED in bf16,
two per 128-partition stack: MT[j+64h, 64q+i] = M_{2q+h}[i,j].  With
lhsT = a [128,128] MT block (stationary operand) both sweeps keep their
results in the PARTITION dim:
  pass A: rhs = ones-blocks [128,2]       -> out[m,n] = rowsums w_p[m]
  pass B: rhs = block-diag w cols [128,4] -> out[m,n] = v_p[m]
LDWEIGHTS/MATMUL pairs pipeline through the PE reorder window (~30ns per
block), so the kernel is DMA-paced: chunks are stored contiguously in
DRAM and streamed in order, and the pipeline is split in column halves
so pass B of half 0 and its tail overlap the DMA of half 1.  The
denominator row v_p[s_p] is gathered with a host mask + ones-matmul;
reciprocal runs on ACT (table preloaded during the DMA window; the DVE
iterative divide on a 1-partition row costs 3.3us).

Sharding: pairs split by s across 8 cores; host sums the partials.
"""

import numpy as np

_N = 64
_NCORES = 8
_NP = 512          # pairs per core
_NQ = 128          # double-stacks (4 pairs each)
_NCHUNK = 8        # DMA chunks of MT
_CW = _NQ * 128 // _NCHUNK   # MT cols per chunk (2048)
_QPC = _NQ // _NCHUNK        # double-stacks per chunk (16)

_cached = {}


def _build_program():
    import concourse.tile as tile
    from concourse import bacc, mybir
    from contextlib import ExitStack

    f32 = mybir.dt.float32
    bf16 = mybir.dt.bfloat16
    AF = mybir.ActivationFunctionType
    nc = bacc.Bacc("TRN2", target_bir_lowering=False, debug=False)
    mt_in = nc.dram_tensor("mt", [_NCHUNK, 128, _CW], bf16,
                           kind="ExternalInput").ap()
    maskd_in = nc.dram_tensor("maskd", [128, _NP], f32, kind="ExternalInput").ap()
    tmt_in = nc.dram_tensor("tmt", [1, _NP], f32, kind="ExternalInput").ap()
    tmb_in = nc.dram_tensor("tmb", [1, _NP], f32, kind="ExternalInput").ap()
    out_dram = nc.dram_tensor("out", [_N, 1], f32, kind="ExternalOutput").ap()

    with tile.TileContext(nc) as tc:
        with ExitStack() as ctx:
            consts = ctx.enter_context(tc.tile_pool(name="consts", bufs=1))
            psum = ctx.enter_context(tc.tile_pool(name="psum", bufs=1, space="PSUM"))

            # ---- small inputs / constants ----
            maskd = consts.tile([128, _NP], f32)
            nc.sync.dma_start(out=maskd[:, :], in_=maskd_in[:, :])
            tmt = consts.tile([1, _NP], f32)
            nc.sync.dma_start(out=tmt[:, :], in_=tmt_in[:, :])
            tmb = consts.tile([1, _NP], f32)
            nc.sync.dma_start(out=tmb[:, :], in_=tmb_in[:, :])

            ones2 = consts.tile([128, 2], bf16)
            nc.vector.memset(ones2[:, :], 0.0)
            nc.vector.memset(ones2[0:64, 0:1], 1.0)
            nc.vector.memset(ones2[64:128, 1:2], 1.0)
            ones128 = consts.tile([128, 1], f32)
            nc.vector.memset(ones128[:, :], 1.0)
            etop = consts.tile([1, 128], f32)
            nc.vector.memset(etop[:, :], 0.0)
            nc.vector.memset(etop[0:1, 0:64], 1.0)
            ebot = consts.tile([1, 128], f32)
            nc.vector.memset(ebot[:, :], 0.0)
            nc.vector.memset(ebot[0:1, 64:128], 1.0)
            L = consts.tile([128, 4 * _NQ], bf16)
            nc.vector.memset(L[:, :], 0.0)
            # hoist the ACT ln/exp table load into the DMA window
            scratch = consts.tile([1, 1], f32)
            nc.scalar.activation(out=scratch[:, :], in_=ones128[0:1, 0:1],
                                 func=AF.Ln)

            # ---- stream MT chunks (contiguous in DRAM, issued in order) ----
            mtc = []
            for d in range(_NCHUNK):
                t = consts.tile([128, _CW], bf16, tag=f"mt{d}")
                nc.sync.dma_start(out=t[:, :], in_=mt_in[d, :, :])
                mtc.append(t)

            WW = psum.tile([128, 2 * _NQ], f32, tag="WW")
            VV = psum.tile([128, _NP], f32, tag="VV")
            DPS = psum.tile([1, _NP], f32, tag="DPS")
            CB = psum.tile([128, _NP], f32, tag="CB")
            WWv = WW[:, :].rearrange("p (q two) -> p q two", two=2)
            Lv = L[:, :].rearrange("p (q four) -> p q four", four=4)
            dmm = consts.tile([128, _NP], f32)
            dinv = consts.tile([1, _NP], f32)
            ct = consts.tile([1, _NP], f32)
            cb = consts.tile([1, _NP], f32)
            cbs = consts.tile([128, _NP], f32)
            vc = consts.tile([128, _NP], f32)
            r1h = []
            for h in (0, 1):
                r1t = consts.tile([128, 1], f32, tag=f"r1{h}")
                r1h.append(r1t)

            def sweepA(h):
                for Q in range(64 * h, 64 * h + 64):
                    d, r = Q // _QPC, Q % _QPC
                    nc.tensor.matmul(
                        out=WW[:, 2 * Q:2 * Q + 2],
                        lhsT=mtc[d][:, 128 * r:128 * r + 128],
                        rhs=ones2[:, :], start=True, stop=True)

            def lbuild(h):
                qs = slice(64 * h, 64 * h + 64)
                nc.vector.tensor_copy(out=Lv[0:64, qs, 0], in_=WWv[0:64, qs, 0])
                nc.vector.tensor_copy(out=Lv[64:128, qs, 1], in_=WWv[0:64, qs, 1])
                nc.vector.tensor_copy(out=Lv[0:64, qs, 2], in_=WWv[64:128, qs, 0])
                nc.vector.tensor_copy(out=Lv[64:128, qs, 3], in_=WWv[64:128, qs, 1])

            def sweepB(h):
                for Q in range(64 * h, 64 * h + 64):
                    d, r = Q // _QPC, Q % _QPC
                    nc.tensor.matmul(
                        out=VV[:, 4 * Q:4 * Q + 4],
                        lhsT=mtc[d][:, 128 * r:128 * r + 128],
                        rhs=L[:, 4 * Q:4 * Q + 4], start=True, stop=True)

            def tail_dve(h):
                sl = slice(256 * h, 256 * h + 256)
                nc.vector.tensor_mul(out=dmm[:, sl], in0=VV[:, sl],
                                     in1=maskd[:, sl])

            def tail_pe_d(h):
                sl = slice(256 * h, 256 * h + 256)
                nc.tensor.matmul(out=DPS[:, sl], lhsT=ones128[:, :],
                                 rhs=dmm[:, sl], start=True, stop=True)

            def tail_coef(h):
                sl = slice(256 * h, 256 * h + 256)
                # 1/d = exp(-ln d) on ACT: d > 0 (Perron), and the DVE
                # iterative divide on a 1-partition row costs 8 cyc/elem.
                nc.scalar.activation(out=dinv[:, sl], in_=DPS[:, sl],
                                     func=AF.Ln)
                nc.scalar.activation(out=dinv[:, sl], in_=dinv[:, sl],
                                     func=AF.Exp, scale=-1.0)
                nc.vector.tensor_mul(out=ct[:, sl], in0=tmt[:, sl],
                                     in1=dinv[:, sl])
                nc.vector.tensor_mul(out=cb[:, sl], in0=tmb[:, sl],
                                     in1=dinv[:, sl])

            def tail_pe_cb(h):
                sl = slice(256 * h, 256 * h + 256)
                nc.tensor.matmul(out=CB[:, sl], lhsT=etop[:, :], rhs=ct[:, sl],
                                 start=True, stop=False)
                nc.tensor.matmul(out=CB[:, sl], lhsT=ebot[:, :], rhs=cb[:, sl],
                                 start=False, stop=True)

            def tail_fin(h):
                sl = slice(256 * h, 256 * h + 256)
                nc.scalar.copy(out=cbs[:, sl], in_=CB[:, sl])
                nc.vector.tensor_mul(out=vc[:, sl], in0=VV[:, sl],
                                     in1=cbs[:, sl])
                nc.vector.tensor_reduce(
                    out=r1h[h][:, :], in_=vc[:, sl],
                    axis=mybir.AxisListType.X, op=mybir.AluOpType.add)

            sweepA(0)
            lbuild(0)
            sweepB(0)
            tail_dve(0)
            sweepA(1)          # PE: runs while half-0 tail DVE work proceeds
            tail_pe_d(0)
            lbuild(1)
            tail_coef(0)
            tail_pe_cb(0)
            sweepB(1)
            tail_fin(0)
            tail_dve(1)
            tail_pe_d(1)
            tail_coef(1)
            tail_pe_cb(1)
            tail_fin(1)

            r1 = consts.tile([128, 1], f32)
            nc.vector.tensor_add(out=r1[:, :], in0=r1h[0][:, :], in1=r1h[1][:, :])
            r1lo = consts.tile([_N, 1], f32)
            nc.scalar.copy(out=r1lo[:, :], in_=r1[64:128, :])
            out_sb = consts.tile([_N, 1], f32)
            nc.vector.tensor_add(out=out_sb[:, :], in0=r1[0:64, :],
                                 in1=r1lo[:, :])
            nc.sync.dma_start(out=out_dram[:, :], in_=out_sb[:, :])
    nc.compile()
    return nc


def _get_program():
    if "nc" not in _cached:
        _cached["nc"] = _build_program()
    return _cached["nc"]


def _build_in_maps(x, weights_t, r_const):
    """Host-side layouts for all 8 cores."""
    import ml_dtypes

    M_all = r_const.reshape(_N * _N, _N, _N)
    i = np.arange(_N)
    r_diag = r_const[i[:, None], i[None, :], i[:, None], i[:, None]]
    T_full = (x * weights_t * r_diag).astype(np.float32)      # [64, 64]

    p = np.arange(_NP)
    b = (p >> 1) & 1                                          # stack-half of pair
    s_loc = p >> 6
    t_loc = p & 63

    in_maps = []
    for c in range(_NCORES):
        Mc = np.asarray(M_all[_NP * c:_NP * (c + 1)], np.float32)  # (p,i,j)
        # MT[j+64h, 64(2Q+b)+i] = Mc[4Q+2b+h, i, j], then chunked contiguously
        mt = (Mc.reshape(_NQ, 2, 2, _N, _N)       # (Q, b, h, i, j)
              .transpose(2, 4, 0, 1, 3)           # (h, j, Q, b, i)
              .reshape(128, _NQ * 128))
        mt = (mt.reshape(128, _NCHUNK, _CW).transpose(1, 0, 2))  # (chunk, p, f)
        mt = np.ascontiguousarray(mt).astype(ml_dtypes.bfloat16)

        maskd = np.zeros((128, _NP), np.float32)
        maskd[64 * b + 8 * c + s_loc, p] = 1.0   # v's node index is GLOBAL s

        Tp = T_full[8 * c + s_loc, t_loc]                     # [512]
        tmt = np.where(b == 0, Tp, 0.0).astype(np.float32)[None, :]
        tmb = np.where(b == 1, Tp, 0.0).astype(np.float32)[None, :]

        in_maps.append({"mt": mt, "maskd": maskd,
                        "tmt": np.ascontiguousarray(tmt),
                        "tmb": np.ascontiguousarray(tmb)})
    return in_maps


def kernel(x, weights_t, weights_r, r_zeros, r_const):
    from concourse.bass_utils import run_bass_kernel_spmd

    x = np.asarray(x, np.float32)
    weights_t = np.asarray(weights_t, np.float32)
    r_const = np.asarray(r_const, np.float32)
    r_zeros_np = np.asarray(r_zeros)
    if np.any(r_zeros_np):
        r_const = (np.asarray(weights_r, np.float32)
                   * r_zeros_np.astype(np.float32) + r_const)

    nc = _get_program()
    in_maps = _build_in_maps(x, weights_t, r_const)
    res = run_bass_kernel_spmd(nc, in_maps, core_ids=list(range(_NCORES)))
    parts = np.stack([r["out"][:, 0] for r in res.results])  # [8, 64]
    return parts.sum(axis=0, dtype=np.float64).astype(np.float32)


# revision 12
# speedup vs baseline: 4.9453x; 1.2872x over previous
"""Trainium2 Bass kernel for nn_DegreePrediction (RBC via batched Perron vectors).

Math: M[s,t] = weights_r*r_zeros + r_const is positive column-stochastic
(columns sum to 1); its eigenvalue-1 right eigenvector is the Perron
vector and rbc[n] = sum_{s,t} T[s,t]/v[s,t,s] * v[s,t,n] is scale-free in
v.  v ~= M^2 @ ones to ~lambda2^2 ~ 0.4% << the 2e-2 gate, so two batched
mat-vec sweeps suffice (no squarings, no transposes).

Layout trick: each core's 512 matrices are uploaded TRANSPOSED in bf16,
two per 128-partition stack: MT[j+64h, 64q+i] = M_{2q+h}[i,j].  With
lhsT = a [128,128] MT block (stationary operand) both sweeps keep their
results in the PARTITION dim:
  pass A: rhs = ones-blocks [128,2]       -> out[m,n] = rowsums w_p[m]
  pass B: rhs = block-diag w cols [128,4] -> out[m,n] = v_p[m]
LDWEIGHTS/MATMUL pairs pipeline through the PE reorder window (~30ns per
block), so the kernel is DMA-paced: chunks are stored contiguously in
DRAM and streamed in order, and the pipeline is split in column halves
so pass B of half 0 and its tail overlap the DMA of half 1.  The
denominator row v_p[s_p] is gathered with a host mask + ones-matmul;
reciprocal runs on ACT (table preloaded during the DMA window; the DVE
iterative divide on a 1-partition row costs 3.3us).

Sharding: pairs split by s across 8 cores; host sums the partials.
"""

import numpy as np

_N = 64
_NCORES = 8
_NP = 512          # pairs per core
_NQ = 128          # double-stacks (4 pairs each)
_NCHUNK = 4        # DMA chunks of MT
_CW = _NQ * 128 // _NCHUNK   # MT cols per chunk (2048)
_QPC = _NQ // _NCHUNK        # double-stacks per chunk (16)

_cached = {}


def _build_program():
    import concourse.tile as tile
    from concourse import bacc, mybir
    from contextlib import ExitStack

    f32 = mybir.dt.float32
    bf16 = mybir.dt.bfloat16
    AF = mybir.ActivationFunctionType
    nc = bacc.Bacc("TRN2", target_bir_lowering=False, debug=False)
    mt_in = nc.dram_tensor("mt", [_NCHUNK, 128, _CW], bf16,
                           kind="ExternalInput").ap()
    maskd_in = nc.dram_tensor("maskd", [128, _NP], f32, kind="ExternalInput").ap()
    tmt_in = nc.dram_tensor("tmt", [1, _NP], f32, kind="ExternalInput").ap()
    tmb_in = nc.dram_tensor("tmb", [1, _NP], f32, kind="ExternalInput").ap()
    e2_in = nc.dram_tensor("e2", [128, _N], f32, kind="ExternalInput").ap()
    out_dram = nc.dram_tensor("out", [1, _N], f32, kind="ExternalOutput").ap()

    with tile.TileContext(nc) as tc:
        with ExitStack() as ctx:
            consts = ctx.enter_context(tc.tile_pool(name="consts", bufs=1))
            psum = ctx.enter_context(tc.tile_pool(name="psum", bufs=1, space="PSUM"))

            # ---- stream MT chunks first (contiguous, in order) ----
            mtc = []
            for d in range(_NCHUNK):
                t = consts.tile([128, _CW], bf16, tag=f"mt{d}")
                nc.sync.dma_start(out=t[:, :], in_=mt_in[d, :, :])
                mtc.append(t)

            # ---- small inputs (tail-only) ----
            maskd = consts.tile([128, _NP], f32)
            nc.sync.dma_start(out=maskd[:, :], in_=maskd_in[:, :])
            tmt = consts.tile([1, _NP], f32)
            nc.sync.dma_start(out=tmt[:, :], in_=tmt_in[:, :])
            tmb = consts.tile([1, _NP], f32)
            nc.sync.dma_start(out=tmb[:, :], in_=tmb_in[:, :])
            e2 = consts.tile([128, _N], f32)
            nc.sync.dma_start(out=e2[:, :], in_=e2_in[:, :])

            ones2 = consts.tile([128, 2], bf16)
            nc.vector.memset(ones2[:, :], 0.0)
            nc.vector.memset(ones2[0:64, 0:1], 1.0)
            nc.vector.memset(ones2[64:128, 1:2], 1.0)
            ones128 = consts.tile([128, 1], bf16)
            nc.vector.memset(ones128[:, :], 1.0)
            one1 = consts.tile([1, 1], f32)
            nc.vector.memset(one1[:, :], 1.0)
            etop = consts.tile([1, 128], bf16)
            nc.vector.memset(etop[:, :], 0.0)
            nc.vector.memset(etop[0:1, 0:64], 1.0)
            ebot = consts.tile([1, 128], bf16)
            nc.vector.memset(ebot[:, :], 0.0)
            nc.vector.memset(ebot[0:1, 64:128], 1.0)
            L = consts.tile([128, 4 * _NQ], bf16)
            nc.vector.memset(L[:, :], 0.0)
            # hoist the ACT ln/exp table load into the DMA window
            scratch = consts.tile([1, 1], f32)
            nc.scalar.activation(out=scratch[:, :], in_=one1[:, :],
                                 func=AF.Ln)

            WW = psum.tile([128, 2 * _NQ], f32, tag="WW")
            VV = psum.tile([128, _NP], f32, tag="VV")
            DPS = psum.tile([1, _NP], f32, tag="DPS")
            CB = psum.tile([128, _NP], f32, tag="CB")
            WWv = WW[:, :].rearrange("p (q two) -> p q two", two=2)
            Lv = L[:, :].rearrange("p (q four) -> p q four", four=4)
            dmm = consts.tile([128, _NP], bf16)
            dinv = consts.tile([1, _NP], f32)
            ct = consts.tile([1, _NP], bf16)
            cb = consts.tile([1, _NP], bf16)
            cbs = consts.tile([128, _NP], f32)
            vc = consts.tile([128, _NP], f32)
            r1h = []
            for h in (0, 1):
                r1t = consts.tile([128, 1], f32, tag=f"r1{h}")
                r1h.append(r1t)

            def sweepA(h):
                for Q in range(64 * h, 64 * h + 64):
                    d, r = Q // _QPC, Q % _QPC
                    nc.tensor.matmul(
                        out=WW[:, 2 * Q:2 * Q + 2],
                        lhsT=mtc[d][:, 128 * r:128 * r + 128],
                        rhs=ones2[:, :], start=True, stop=True)

            def lbuild(h):
                qs = slice(64 * h, 64 * h + 64)
                nc.vector.tensor_copy(out=Lv[0:64, qs, 0], in_=WWv[0:64, qs, 0])
                nc.vector.tensor_copy(out=Lv[64:128, qs, 1], in_=WWv[0:64, qs, 1])
                nc.vector.tensor_copy(out=Lv[0:64, qs, 2], in_=WWv[64:128, qs, 0])
                nc.vector.tensor_copy(out=Lv[64:128, qs, 3], in_=WWv[64:128, qs, 1])

            def sweepB(h):
                for Q in range(64 * h, 64 * h + 64):
                    d, r = Q // _QPC, Q % _QPC
                    nc.tensor.matmul(
                        out=VV[:, 4 * Q:4 * Q + 4],
                        lhsT=mtc[d][:, 128 * r:128 * r + 128],
                        rhs=L[:, 4 * Q:4 * Q + 4], start=True, stop=True)

            def tail_dve(h):
                sl = slice(256 * h, 256 * h + 256)
                nc.vector.tensor_mul(out=dmm[:, sl], in0=VV[:, sl],
                                     in1=maskd[:, sl])

            def tail_pe_d(h):
                sl = slice(256 * h, 256 * h + 256)
                nc.tensor.matmul(out=DPS[:, sl], lhsT=ones128[:, :],
                                 rhs=dmm[:, sl], start=True, stop=True)

            def tail_coef(h):
                sl = slice(256 * h, 256 * h + 256)
                # 1/d = exp(-ln d) on ACT: d > 0 (Perron), and the DVE
                # iterative divide on a 1-partition row costs 8 cyc/elem.
                nc.scalar.activation(out=dinv[:, sl], in_=DPS[:, sl],
                                     func=AF.Ln)
                nc.scalar.activation(out=dinv[:, sl], in_=dinv[:, sl],
                                     func=AF.Exp, scale=-1.0)
                nc.vector.tensor_mul(out=ct[:, sl], in0=tmt[:, sl],
                                     in1=dinv[:, sl])
                nc.vector.tensor_mul(out=cb[:, sl], in0=tmb[:, sl],
                                     in1=dinv[:, sl])

            def tail_pe_cb(h):
                sl = slice(256 * h, 256 * h + 256)
                nc.tensor.matmul(out=CB[:, sl], lhsT=etop[:, :], rhs=ct[:, sl],
                                 start=True, stop=False)
                nc.tensor.matmul(out=CB[:, sl], lhsT=ebot[:, :], rhs=cb[:, sl],
                                 start=False, stop=True)

            def tail_fin(h):
                sl = slice(256 * h, 256 * h + 256)
                nc.scalar.copy(out=cbs[:, sl], in_=CB[:, sl])
                nc.vector.tensor_mul(out=vc[:, sl], in0=VV[:, sl],
                                     in1=cbs[:, sl])
                nc.vector.tensor_reduce(
                    out=r1h[h][:, :], in_=vc[:, sl],
                    axis=mybir.AxisListType.X, op=mybir.AluOpType.add)

            sweepA(0)
            lbuild(0)
            sweepB(0)
            tail_dve(0)
            sweepA(1)          # PE: runs while half-0 tail DVE work proceeds
            tail_pe_d(0)
            lbuild(1)
            tail_coef(0)
            tail_pe_cb(0)
            sweepB(1)
            tail_fin(0)
            tail_dve(1)
            tail_pe_d(1)
            tail_coef(1)
            tail_pe_cb(1)
            tail_fin(1)

            r1 = consts.tile([128, 1], f32)
            nc.vector.tensor_add(out=r1[:, :], in0=r1h[0][:, :], in1=r1h[1][:, :])
            # fold halves AND transpose to a row in one matmul:
            # FR[0,n] = sum_k r1[k]*E2[k,n] = r1[n] + r1[64+n]
            FR = psum.tile([1, _N], f32, tag="FR")
            nc.tensor.matmul(out=FR[:, :], lhsT=r1[:, :], rhs=e2[:, :],
                             start=True, stop=True)
            out_sb = consts.tile([1, _N], f32)
            nc.scalar.copy(out=out_sb[:, :], in_=FR[:, :])
            nc.sync.dma_start(out=out_dram[:, :], in_=out_sb[:, :])
    nc.compile()
    return nc


def _get_program():
    if "nc" not in _cached:
        _cached["nc"] = _build_program()
    return _cached["nc"]


def _build_in_maps(x, weights_t, r_const):
    """Host-side layouts for all 8 cores."""
    import ml_dtypes

    M_all = r_const.reshape(_N * _N, _N, _N)
    i = np.arange(_N)
    r_diag = r_const[i[:, None], i[None, :], i[:, None], i[:, None]]
    T_full = (x * weights_t * r_diag).astype(np.float32)      # [64, 64]

    e2 = np.zeros((128, _N), np.float32)
    e2[np.arange(128), np.arange(128) % _N] = 1.0

    p = np.arange(_NP)
    b = (p >> 1) & 1                                          # stack-half of pair
    s_loc = p >> 6
    t_loc = p & 63

    in_maps = []
    for c in range(_NCORES):
        Mc = np.asarray(M_all[_NP * c:_NP * (c + 1)], np.float32)  # (p,i,j)
        # MT[j+64h, 64(2Q+b)+i] = Mc[4Q+2b+h, i, j], then chunked contiguously
        mt = (Mc.reshape(_NQ, 2, 2, _N, _N)       # (Q, b, h, i, j)
              .transpose(2, 4, 0, 1, 3)           # (h, j, Q, b, i)
              .reshape(128, _NQ * 128))
        mt = (mt.reshape(128, _NCHUNK, _CW).transpose(1, 0, 2))  # (chunk, p, f)
        mt = np.ascontiguousarray(mt).astype(ml_dtypes.bfloat16)

        maskd = np.zeros((128, _NP), np.float32)
        maskd[64 * b + 8 * c + s_loc, p] = 1.0   # v's node index is GLOBAL s

        Tp = T_full[8 * c + s_loc, t_loc]                     # [512]
        tmt = np.where(b == 0, Tp, 0.0).astype(np.float32)[None, :]
        tmb = np.where(b == 1, Tp, 0.0).astype(np.float32)[None, :]

        in_maps.append({"mt": mt, "maskd": maskd,
                        "tmt": np.ascontiguousarray(tmt),
                        "tmb": np.ascontiguousarray(tmb), "e2": e2})
    return in_maps


def kernel(x, weights_t, weights_r, r_zeros, r_const):
    from concourse.bass_utils import run_bass_kernel_spmd

    x = np.asarray(x, np.float32)
    weights_t = np.asarray(weights_t, np.float32)
    r_const = np.asarray(r_const, np.float32)
    r_zeros_np = np.asarray(r_zeros)
    if np.any(r_zeros_np):
        r_const = (np.asarray(weights_r, np.float32)
                   * r_zeros_np.astype(np.float32) + r_const)

    nc = _get_program()
    in_maps = _build_in_maps(x, weights_t, r_const)
    res = run_bass_kernel_spmd(nc, in_maps, core_ids=list(range(_NCORES)))
    parts = np.stack([r["out"][0, :] for r in res.results])  # [8, 64]
    return parts.sum(axis=0, dtype=np.float64).astype(np.float32)
